# revision 13
# baseline (speedup 1.0000x reference)
"""Trainium2 Bass kernel for nn_ExpandedResolventFMNet.

Mathematical reformulation (validated in fp64 against the jax reference):

The reference builds kron(A.T, My) [8192x4096], its Gram [4096^2], resolvent
kron masks, and solves a dense 4096x4096 system.  All of that collapses:

  first        = kron(A A^T, G),              G = My^T My
  second       = kron-sum of 64x64 factors; with X = Mx W the full system is
  M(W)         = S~ W G + LMBDA * sum_d Dd*( (Dd*W) G ) = R~    (* = Hadamard)
  S~           = Mx^T (A A^T) Mx
  R~           = Mx^T A Bc^T My
  Dd           = resolvent-mask difference matrices (64x64)
  output C     = (Mx W)^T

The 4096x4096 operator kron(S~,G)+LMBDA*blockdiag is SPD with cond ~3e2; PCG
with the exact-kron preconditioner P^-1 = kron(S~^-1, G^-1) (applied as two
64x64 matmuls) converges to the fp32 floor in <=14 iterations.  The device
runs the transposed system in Y = W^T so every matmul has an SBUF-resident
stationary (lhsT) operand:

  M'(Y) = G Y S~ + sum_d DdT * (G (DdT * Y)),   C = Y Mx^T

S~^-1 and G^-1 are produced on-device by Newton-Schulz iteration.
sqrt(LMBDA) is folded into DdT.
"""

import numpy as np

import concourse.bass as bass
import concourse.bacc as bacc
import concourse.mybir as mybir
from concourse.bass_utils import run_bass_kernel_spmd
from concourse.masks import make_identity
from concourse.tile import TileContext

F32 = mybir.dt.float32
K = 64          # spectral basis size
C = 128         # feature channels
V = 5000        # vertices
CHUNK = 125     # v-contraction tile (partition dim)
N_CORES = 8
N_ITERS = 16
NEWTON_STEPS = 8
SQRT_LMBDA = 10.0

SHARD = False   # False: all cores run the full problem redundantly (no comm)
                # True:  v-dim sharded projections + AllReduce

_PROGRAM_CACHE = {}


def build_program(shard: bool):
    nc = bacc.Bacc("TRN2", num_devices=N_CORES)
    v_local = V // N_CORES if shard else V          # 625 or 5000
    n_chunks = v_local // CHUNK                     # 5 or 40

    fx_d = nc.dram_tensor("fx", [v_local, C], F32, kind="ExternalInput")
    fy_d = nc.dram_tensor("fy", [v_local, C], F32, kind="ExternalInput")
    pxT_d = nc.dram_tensor("pxT", [v_local, K], F32, kind="ExternalInput")
    pyT_d = nc.dram_tensor("pyT", [v_local, K], F32, kind="ExternalInput")
    mx_d = nc.dram_tensor("mx", [K, K], F32, kind="ExternalInput")
    my_d = nc.dram_tensor("my", [K, K], F32, kind="ExternalInput")
    mxT_d = nc.dram_tensor("mxT", [K, K], F32, kind="ExternalInput")
    myT_d = nc.dram_tensor("myT", [K, K], F32, kind="ExternalInput")
    ev_d = nc.dram_tensor("ev", [1, 2 * K], F32, kind="ExternalInput")
    out_d = nc.dram_tensor("out", [K, K], F32, kind="ExternalOutput")

    if shard:
        cc_in = nc.dram_tensor("cc_in", [C, 2 * K], F32)
        cc_out = nc.dram_tensor("cc_out", [C, 2 * K], F32, addr_space="Shared")

    with TileContext(nc) as tc:
        with (
            tc.tile_pool(name="big", bufs=1) as bp,
            tc.tile_pool(name="persist", bufs=1) as sp,
            tc.tile_pool(name="work", bufs=2) as wp,
            tc.tile_pool(name="psum", bufs=2, space="PSUM") as pp,
        ):
            # rotating psum tags: 3 tags x bufs=2 -> 6 banks (+pacc 2 = 8)
            _ps_state = {"i": 0}

            def ps_tile(shape):
                i = _ps_state["i"]
                _ps_state["i"] += 1
                return pp.tile(shape, F32, tag=f"ps{i % 3}", name=f"pst{i}")

            def sb_copy(src_psum, shape, pool, tag, engine="vector"):
                t = pool.tile(shape, F32, tag=tag)
                if engine == "vector":
                    nc.vector.tensor_copy(t, src_psum)
                else:
                    nc.scalar.copy(t, src_psum)
                return t

            # ---------------- input DMA ----------------
            fx_t = bp.tile([CHUNK, n_chunks, C], F32)
            fy_t = bp.tile([CHUNK, n_chunks, C], F32)
            pxT_t = bp.tile([CHUNK, n_chunks, K], F32)
            pyT_t = bp.tile([CHUNK, n_chunks, K], F32)
            for n in range(n_chunks):
                lo = n * CHUNK
                nc.sync.dma_start(fx_t[:, n, :], fx_d[lo:lo + CHUNK, :])
                nc.sync.dma_start(fy_t[:, n, :], fy_d[lo:lo + CHUNK, :])
                nc.sync.dma_start(pxT_t[:, n, :], pxT_d[lo:lo + CHUNK, :])
                nc.sync.dma_start(pyT_t[:, n, :], pyT_d[lo:lo + CHUNK, :])
            mx_s = sp.tile([K, K], F32)
            my_s = sp.tile([K, K], F32)
            mxT_s = sp.tile([K, K], F32)
            myT_s = sp.tile([K, K], F32)
            ev_t = sp.tile([1, 2 * K], F32)
            nc.sync.dma_start(mx_s, mx_d[:, :])
            nc.sync.dma_start(my_s, my_d[:, :])
            nc.sync.dma_start(mxT_s, mxT_d[:, :])
            nc.sync.dma_start(myT_s, myT_d[:, :])
            nc.sync.dma_start(ev_t, ev_d[:, :])

            ident = sp.tile([C, C], F32)
            make_identity(nc, ident)
            id64 = ident[0:K, 0:K]
            ones_row = sp.tile([1, K], F32)
            nc.vector.memset(ones_row, 1.0)
            ones_col = sp.tile([K, 1], F32)
            nc.vector.memset(ones_col, 1.0)

            def bcast_scalar(s_sb, tag):
                """[1,1] SBUF -> [K,1] SBUF (broadcast via PE outer product)."""
                b_p = ps_tile([K, 1])
                nc.tensor.matmul(b_p, ones_row, s_sb)
                b_s = wp.tile([K, 1], F32, tag=f"{tag}_bs", name=f"{tag}_bs")
                nc.vector.tensor_copy(b_s, b_p)
                return b_s

            def psum_reduce(acc_sb, tag):
                """[K,1] SBUF -> scalar [1,1] SBUF (sum over partitions)."""
                s_p = ps_tile([1, 1])
                nc.tensor.matmul(s_p, acc_sb, ones_col)
                s_s = wp.tile([1, 1], F32, tag=f"{tag}_ss", name=f"{tag}_ss")
                nc.vector.tensor_copy(s_s, s_p)
                return s_s

            # ---------------- projections: AT = fx^T pxT, ByT = fy^T pyT ----
            # matmul(out, lhsT, rhs) = lhsT.T @ rhs, contraction over v-chunk
            with tc.tile_pool(name="pacc", bufs=1, space="PSUM") as pacc:
                at_p = pacc.tile([C, K], F32)    # A^T = (Px fx)^T  [C,K]
                byt_p = pacc.tile([C, K], F32)   # By^T             [C,K]
                for n in range(n_chunks):
                    nc.tensor.matmul(at_p, fx_t[:, n, :], pxT_t[:, n, :],
                                     start=(n == 0), stop=(n == n_chunks - 1))
                for n in range(n_chunks):
                    nc.tensor.matmul(byt_p, fy_t[:, n, :], pyT_t[:, n, :],
                                     start=(n == 0), stop=(n == n_chunks - 1))

                if shard:
                    part_s = sp.tile([C, 2 * K], F32)
                    nc.vector.tensor_copy(part_s[:, 0:K], at_p)
                    nc.vector.tensor_copy(part_s[:, K:2 * K], byt_p)
                    nc.sync.dma_start(cc_in[:, :], part_s)
                    nc.gpsimd.collective_compute(
                        "AllReduce",
                        mybir.AluOpType.add,
                        replica_groups=[list(range(N_CORES))],
                        ins=[cc_in[:, :]],
                        outs=[cc_out[:, :]],
                    )
                    full_s = sp.tile([C, 2 * K], F32)
                    nc.sync.dma_start(full_s, cc_out[:, :])
                    at_s = full_s[:, 0:K]
                    byt_s = full_s[:, K:2 * K]
                else:
                    at_s = sb_copy(at_p, [C, K], sp, "at_s")
                    byt_s = sb_copy(byt_p, [C, K], sp, "byt_s")

            # ---------------- small-matrix chain ----------------
            # BcT = By^T My^T : matmul(lhsT=By, rhs=MyT);  By = transpose(ByT)
            by_p = ps_tile([K, C])
            nc.tensor.transpose(by_p, byt_s, ident)
            by_s = sb_copy(by_p, [K, C], sp, "by_s")
            bct_p = ps_tile([C, K])
            nc.tensor.matmul(bct_p, by_s, myT_s)
            bct_s = sb_copy(bct_p, [C, K], sp, "bct_s")

            # A [K,C] = transpose(AT)
            a_p = ps_tile([K, C])
            nc.tensor.transpose(a_p, at_s, ident)
            a_s = sb_copy(a_p, [K, C], sp, "a_s")

            # S~ = Mx^T (A A^T) Mx
            sa_p = ps_tile([K, K])
            nc.tensor.matmul(sa_p, at_s, at_s)          # A A^T
            sa_s = sb_copy(sa_p, [K, K], sp, "sa_s")
            h1_p = ps_tile([K, K])
            nc.tensor.matmul(h1_p, mx_s, sa_s)          # Mx^T S_A
            h1_s = sb_copy(h1_p, [K, K], sp, "h1_s")
            h1t_p = ps_tile([K, K])
            nc.tensor.transpose(h1t_p, h1_s, id64)      # S_A Mx
            h1t_s = sb_copy(h1t_p, [K, K], sp, "h1t_s")
            st_p = ps_tile([K, K])
            nc.tensor.matmul(st_p, mx_s, h1t_s)         # Mx^T S_A Mx
            st_s = sb_copy(st_p, [K, K], sp, "st_s")

            # G = My^T My
            g_p = ps_tile([K, K])
            nc.tensor.matmul(g_p, my_s, my_s)
            g_s = sb_copy(g_p, [K, K], sp, "g_s")

            # RHS' = My^T Bc A^T Mx  (= R~^T)
            z1_p = ps_tile([C, K])
            nc.tensor.matmul(z1_p, a_s, mx_s)           # A^T Mx [C,K]
            z1_s = sb_copy(z1_p, [C, K], sp, "z1_s")
            z2_p = ps_tile([K, K])
            nc.tensor.matmul(z2_p, bct_s, z1_s)         # Bc (A^T Mx)
            z2_s = sb_copy(z2_p, [K, K], sp, "z2_s")
            rhs_p = ps_tile([K, K])
            nc.tensor.matmul(rhs_p, my_s, z2_s)         # My^T Z2
            r_s = sp.tile([K, K], F32)                  # CG residual
            nc.vector.tensor_copy(r_s, rhs_p)

            # ---------------- resolvent masks ----------------
            # ev = [ex | ey] in one row; t = ev/max(ev); im = 1/(1+t);
            # re = sqrt(t)*im; both scaled by sqrt(LMBDA)
            evmax = sp.tile([1, 1], F32)
            nc.vector.tensor_reduce(evmax, ev_t, mybir.AxisListType.X,
                                    mybir.AluOpType.max)
            evrec = sp.tile([1, 1], F32)
            nc.vector.reciprocal(evrec, evmax)
            t_t = sp.tile([1, 2 * K], F32)
            nc.vector.tensor_scalar_mul(t_t, ev_t, evrec)
            tp1 = sp.tile([1, 2 * K], F32)
            nc.vector.tensor_scalar_add(tp1, t_t, 1.0)
            im_t = sp.tile([1, 2 * K], F32)
            nc.vector.reciprocal(im_t, tp1)
            sq_t = sp.tile([1, 2 * K], F32)
            nc.scalar.sqrt(sq_t, t_t)
            re_t = sp.tile([1, 2 * K], F32)
            nc.vector.tensor_mul(re_t, sq_t, im_t)
            nc.vector.tensor_scalar_mul(re_t, re_t, SQRT_LMBDA)
            nc.vector.tensor_scalar_mul(im_t, im_t, SQRT_LMBDA)

            # D1T[a,i] = re2[a] - re1[i]; D2T likewise from im
            d_s = []
            for idx, src in enumerate((re_t, im_t)):
                pa = ps_tile([K, K])
                nc.tensor.matmul(pa, src[0:1, K:2 * K], ones_row)  # v2[p]
                pb = ps_tile([K, K])
                nc.tensor.matmul(pb, ones_row, src[0:1, 0:K])      # v1[f]
                ta = sb_copy(pa, [K, K], sp, f"dta{idx}")
                dt = sp.tile([K, K], F32, tag=f"d{idx}t_s")
                nc.vector.tensor_sub(dt, ta, pb)
                d_s.append(dt)
            d1t_s, d2t_s = d_s

            # ---------------- Newton-Schulz inverses ----------------
            def newton_inverse(mat_s, tag):
                rs = sp.tile([K, 1], F32, tag=f"{tag}_rs")
                nc.vector.tensor_reduce(rs, mat_s, mybir.AxisListType.X,
                                        mybir.AluOpType.add,
                                        apply_absolute_value=True)
                # max over partitions: transpose to a row, reduce, invert, bcast
                rst_p = ps_tile([1, K])
                nc.tensor.transpose(rst_p, rs, id64)
                rst_s = sp.tile([1, K], F32, tag=f"{tag}_rst")
                nc.vector.tensor_copy(rst_s, rst_p)
                mxv = sp.tile([1, 1], F32, tag=f"{tag}_mxv")
                nc.vector.tensor_reduce(mxv, rst_s, mybir.AxisListType.X,
                                        mybir.AluOpType.max)
                al0 = sp.tile([1, 1], F32, tag=f"{tag}_al0")
                nc.vector.reciprocal(al0, mxv)
                al = bcast_scalar(al0, f"{tag}_al")
                x_s = sp.tile([K, K], F32, tag=f"{tag}_x0")
                nc.vector.tensor_scalar_mul(x_s, id64, al)
                for it in range(NEWTON_STEPS):
                    t1 = ps_tile([K, K])
                    nc.tensor.matmul(t1, mat_s, x_s)          # S X (S sym)
                    t1s = wp.tile([K, K], F32, tag=f"{tag}_t1s")
                    nc.vector.tensor_copy(t1s, t1)
                    t2 = ps_tile([K, K])
                    nc.tensor.matmul(t2, x_s, t1s)            # X (S X) (X sym)
                    xn = sp.tile([K, K], F32, tag=f"{tag}_x{it + 1}")
                    nc.vector.scalar_tensor_tensor(
                        xn, x_s, 2.0, t2,
                        op0=mybir.AluOpType.mult,
                        op1=mybir.AluOpType.subtract)         # 2X - XSX
                    x_s = xn
                return x_s

            si_s = newton_inverse(st_s, "si")
            gi_s = newton_inverse(g_s, "gi")

            # ---------------- PCG (transposed space) ----------------
            y_s = sp.tile([K, K], F32)
            nc.vector.memset(y_s, 0.0)
            pstack = sp.tile([K, 3 * K], F32)

            def precond(r_tile):
                """z = (Gi r) Si ; returns PSUM AP of z."""
                u_p = ps_tile([K, K])
                nc.tensor.matmul(u_p, gi_s, r_tile)           # Gi r (Gi sym)
                u_s = wp.tile([K, K], F32, tag="pc_us")
                nc.vector.tensor_copy(u_s, u_p)
                ut_p = ps_tile([K, K])
                nc.tensor.transpose(ut_p, u_s, id64)
                ut_s = wp.tile([K, K], F32, tag="pc_uts")
                nc.vector.tensor_copy(ut_s, ut_p)
                z_p = ps_tile([K, K])
                nc.tensor.matmul(z_p, ut_s, si_s)             # (Gi r) Si
                return z_p

            def dot(a_ap, b_ap, tag):
                # NOTE: tensor_tensor_reduce hard-faults this HW path; use
                # mul + reduce instead.
                prod = wp.tile([K, K], F32, tag="dot_dm", name="dot_dm")
                nc.vector.tensor_mul(prod, a_ap, b_ap)
                acc = wp.tile([K, 1], F32, tag=f"{tag}_acc", name=f"{tag}_acc")
                nc.vector.tensor_reduce(acc, prod, mybir.AxisListType.X,
                                        mybir.AluOpType.add)
                s_s = psum_reduce(acc, tag)
                return bcast_scalar(s_s, tag)

            z_p = precond(r_s)
            nc.vector.tensor_copy(pstack[:, 0:K], z_p)
            rz = dot(r_s, z_p, "rz")

            for it in range(N_ITERS):
                p_ap = pstack[:, 0:K]
                # ---- q = M'(p) = G p S~ + sum_d DdT*(G (DdT*p)) ----
                nc.vector.tensor_mul(pstack[:, K:2 * K], d1t_s, p_ap)
                nc.vector.tensor_mul(pstack[:, 2 * K:3 * K], d2t_s, p_ap)
                t1 = ps_tile([K, 3 * K])
                nc.tensor.matmul(t1, g_s, pstack)   # [G p | G W1 | G W2]
                gy_s = wp.tile([K, K], F32, tag="mv_gys")
                nc.scalar.copy(gy_s, t1[:, 0:K])
                gyt_p = ps_tile([K, K])
                nc.tensor.transpose(gyt_p, gy_s, id64)
                gyt_s = wp.tile([K, K], F32, tag="mv_gyts")
                nc.scalar.copy(gyt_s, gyt_p)
                t2 = ps_tile([K, K])
                nc.tensor.matmul(t2, gyt_s, st_s)   # (G p) S~
                m1_s = wp.tile([K, K], F32, tag="mv_m1")
                nc.vector.tensor_mul(m1_s, d1t_s, t1[:, K:2 * K])
                m2_s = wp.tile([K, K], F32, tag="mv_m2")
                nc.vector.tensor_mul(m2_s, d2t_s, t1[:, 2 * K:3 * K])
                ms_s = wp.tile([K, K], F32, tag="mv_ms")
                nc.vector.tensor_add(ms_s, m1_s, m2_s)
                q_s = wp.tile([K, K], F32, tag="mv_q")
                nc.vector.tensor_add(q_s, ms_s, t2)

                # ---- alpha = rz / <p,q> ----
                pq = dot(p_ap, q_s, "pq")
                rpq = wp.tile([K, 1], F32, tag="rpq")
                nc.vector.reciprocal(rpq, pq)
                al = wp.tile([K, 1], F32, tag="al")
                nc.vector.tensor_mul(al, rz, rpq)
                aln = wp.tile([K, 1], F32, tag="aln")
                nc.vector.tensor_scalar_mul(aln, al, -1.0)

                # ---- y += alpha p ; r -= alpha q ----
                nc.vector.scalar_tensor_tensor(
                    y_s, p_ap, al, y_s,
                    op0=mybir.AluOpType.mult, op1=mybir.AluOpType.add)
                nc.vector.scalar_tensor_tensor(
                    r_s, q_s, aln, r_s,
                    op0=mybir.AluOpType.mult, op1=mybir.AluOpType.add)

                if it == N_ITERS - 1:
                    break

                # ---- z = P^-1 r ; beta = <r,z>/rz ; p = z + beta p ----
                z_p = precond(r_s)
                rz_new = dot(r_s, z_p, "rzn")
                rzrec = wp.tile([K, 1], F32, tag="rzrec")
                nc.vector.reciprocal(rzrec, rz)
                bt = wp.tile([K, 1], F32, tag="bt")
                nc.vector.tensor_mul(bt, rz_new, rzrec)
                nc.vector.scalar_tensor_tensor(
                    pstack[:, 0:K], pstack[:, 0:K], bt, z_p,
                    op0=mybir.AluOpType.mult, op1=mybir.AluOpType.add)
                rz = rz_new

            # ---------------- output: C = Y Mx^T ----------------
            yt_p = ps_tile([K, K])
            nc.tensor.transpose(yt_p, y_s, id64)
            yt_s = wp.tile([K, K], F32, tag="yt_s")
            nc.vector.tensor_copy(yt_s, yt_p)
            c_p = ps_tile([K, K])
            nc.tensor.matmul(c_p, yt_s, mxT_s)      # Y Mx^T
            c_s = wp.tile([K, K], F32, tag="c_s")
            nc.vector.tensor_copy(c_s, c_p)
            nc.sync.dma_start(out_d[:, :], c_s)

    nc.finalize()
    return nc


def get_program(shard: bool):
    if shard not in _PROGRAM_CACHE:
        _PROGRAM_CACHE[shard] = build_program(shard)
    return _PROGRAM_CACHE[shard]


def make_in_maps(inputs, shard: bool):
    fx = np.ascontiguousarray(np.asarray(inputs["feat_x"], np.float32)[0])
    fy = np.ascontiguousarray(np.asarray(inputs["feat_y"], np.float32)[0])
    pxT = np.ascontiguousarray(np.asarray(inputs["evecs_trans_x"], np.float32)[0].T)
    pyT = np.ascontiguousarray(np.asarray(inputs["evecs_trans_y"], np.float32)[0].T)
    mx = np.ascontiguousarray(np.asarray(inputs["sqrtMk_x"], np.float32)[0])
    my = np.ascontiguousarray(np.asarray(inputs["sqrtMk_y"], np.float32)[0])
    ev = np.ascontiguousarray(np.concatenate([
        np.asarray(inputs["evals_x"], np.float32)[0],
        np.asarray(inputs["evals_y"], np.float32)[0],
    ])[None, :])
    small = {
        "mx": mx, "my": my,
        "mxT": np.ascontiguousarray(mx.T),
        "myT": np.ascontiguousarray(my.T),
        "ev": ev,
    }
    in_maps = []
    for c in range(N_CORES):
        if shard:
            lo, hi = c * (V // N_CORES), (c + 1) * (V // N_CORES)
            m = {"fx": fx[lo:hi], "fy": fy[lo:hi],
                 "pxT": pxT[lo:hi], "pyT": pyT[lo:hi]}
        else:
            m = {"fx": fx, "fy": fy, "pxT": pxT, "pyT": pyT}
        m.update(small)
        in_maps.append(m)
    return in_maps


def kernel(**inputs) -> np.ndarray:
    nc = get_program(SHARD)
    in_maps = make_in_maps(inputs, SHARD)
    res = run_bass_kernel_spmd(nc, in_maps, core_ids=list(range(N_CORES)))
    out = np.asarray(res.results[0]["out"], dtype=np.float32)
    return out[None]


# revision 15
# speedup vs baseline: 1.5493x; 1.5493x over previous
"""Trainium2 Bass kernel for nn_ExpandedResolventFMNet.

Mathematical reformulation (validated in fp64 against the jax reference):

The reference builds kron(A.T, My) [8192x4096], its Gram [4096^2], resolvent
kron masks, and solves a dense 4096x4096 system.  All of that collapses:

  first        = kron(A A^T, G),              G = My^T My
  second       = kron-sum of 64x64 factors; with X = Mx W the full system is
  M(W)         = S~ W G + LMBDA * sum_d Dd*( (Dd*W) G ) = R~    (* = Hadamard)
  S~           = Mx^T (A A^T) Mx
  R~           = Mx^T A Bc^T My
  Dd           = resolvent-mask difference matrices (64x64)
  output C     = (Mx W)^T

The 4096x4096 operator kron(S~,G)+LMBDA*blockdiag is SPD with cond ~3e2; PCG
with the exact-kron preconditioner P^-1 = kron(S~^-1, G^-1) (applied as two
64x64 matmuls) converges to the fp32 floor in <=14 iterations.  The device
runs the transposed system in Y = W^T:

  M'(Y) = G Y S~ + sum_d DdT * (G (DdT * Y)),   C = Y Mx^T

and exploits symmetry so that every matmul is transpose-free:
  (G p)^T   = mm(lhsT=p,  rhs=G)     [G symmetric]
  (G p) S~  = mm(lhsT=(G p)^T, rhs=S~)
  (Gi r)^T  = mm(lhsT=r,  rhs=Gi)
  (Gi r) Si = mm(lhsT=(Gi r)^T, rhs=Si)

S~^-1 and G^-1 are produced on-device by Newton-Schulz iteration.
sqrt(LMBDA) is folded into DdT.  Work is sharded over 8 cores for the
V=5000 projections (AllReduce of the 64KB partials); the small solve runs
redundantly on every core.
"""

import numpy as np

import concourse.bacc as bacc
import concourse.mybir as mybir
from concourse.bass_isa import ReduceOp
from concourse.bass_utils import run_bass_kernel_spmd
from concourse.masks import make_identity
from concourse.tile import TileContext

F32 = mybir.dt.float32
K = 64          # spectral basis size
C = 128         # feature channels
V = 5000        # vertices
CHUNK = 125     # v-contraction tile (partition dim)
N_CORES = 8
N_ITERS = 14
NEWTON_STEPS_S = 8
NEWTON_STEPS_G = 4
SQRT_LMBDA = 10.0

SHARD = True    # shard projections over cores + AllReduce partials

_PROGRAM_CACHE = {}


def build_program(shard: bool):
    nc = bacc.Bacc("TRN2", num_devices=N_CORES)
    v_local = V // N_CORES if shard else V          # 625 or 5000
    n_chunks = v_local // CHUNK                     # 5 or 40

    fx_d = nc.dram_tensor("fx", [v_local, C], F32, kind="ExternalInput")
    fy_d = nc.dram_tensor("fy", [v_local, C], F32, kind="ExternalInput")
    pxT_d = nc.dram_tensor("pxT", [v_local, K], F32, kind="ExternalInput")
    pyT_d = nc.dram_tensor("pyT", [v_local, K], F32, kind="ExternalInput")
    mx_d = nc.dram_tensor("mx", [K, K], F32, kind="ExternalInput")
    my_d = nc.dram_tensor("my", [K, K], F32, kind="ExternalInput")
    mxT_d = nc.dram_tensor("mxT", [K, K], F32, kind="ExternalInput")
    myT_d = nc.dram_tensor("myT", [K, K], F32, kind="ExternalInput")
    ev_d = nc.dram_tensor("ev", [1, 2 * K], F32, kind="ExternalInput")
    out_d = nc.dram_tensor("out", [K, K], F32, kind="ExternalOutput")

    if shard:
        cc_in = nc.dram_tensor("cc_in", [C, 2 * K], F32)
        cc_out = nc.dram_tensor("cc_out", [C, 2 * K], F32, addr_space="Shared")
        ccw_in = nc.dram_tensor("ccw_in", [1, K], F32)
        ccw_out = nc.dram_tensor("ccw_out", [1, K], F32, addr_space="Shared")

    with TileContext(nc) as tc:
        with (
            tc.tile_pool(name="big", bufs=1) as bp,
            tc.tile_pool(name="persist", bufs=1) as sp,
            tc.tile_pool(name="work", bufs=2) as wp,
            tc.tile_pool(name="psum", bufs=2, space="PSUM") as pp,
        ):
            # rotating psum tags: 3 tags x bufs=2 -> 6 banks (+proj acc 2 = 8)
            _ps_state = {"i": 0}

            def ps_tile(shape):
                i = _ps_state["i"]
                _ps_state["i"] += 1
                return pp.tile(shape, F32, tag=f"ps{i % 3}", name=f"pst{i}")

            def sb_copy(src_psum, shape, pool, tag, engine="vector"):
                t = pool.tile(shape, F32, tag=tag, name=tag)
                if engine == "vector":
                    nc.vector.tensor_copy(t, src_psum)
                else:
                    nc.scalar.copy(t, src_psum)
                return t

            # ---------------- input DMA (one DMA per big tensor) ------------
            fx_t = bp.tile([CHUNK, n_chunks, C], F32)
            fy_t = bp.tile([CHUNK, n_chunks, C], F32)
            pxT_t = bp.tile([CHUNK, n_chunks, K], F32)
            pyT_t = bp.tile([CHUNK, n_chunks, K], F32)
            nc.sync.dma_start(
                fx_t, fx_d.rearrange("(n p) c -> p n c", p=CHUNK))
            nc.sync.dma_start(
                fy_t, fy_d.rearrange("(n p) c -> p n c", p=CHUNK))
            nc.sync.dma_start(
                pxT_t, pxT_d.rearrange("(n p) c -> p n c", p=CHUNK))
            nc.sync.dma_start(
                pyT_t, pyT_d.rearrange("(n p) c -> p n c", p=CHUNK))
            mx_s = sp.tile([K, K], F32)
            my_s = sp.tile([K, K], F32)
            mxT_s = sp.tile([K, K], F32)
            myT_s = sp.tile([K, K], F32)
            ev_t = sp.tile([1, 2 * K], F32)
            nc.sync.dma_start(mx_s, mx_d[:, :])
            nc.sync.dma_start(my_s, my_d[:, :])
            nc.sync.dma_start(mxT_s, mxT_d[:, :])
            nc.sync.dma_start(myT_s, myT_d[:, :])
            nc.sync.dma_start(ev_t, ev_d[:, :])

            ident = sp.tile([C, C], F32)
            make_identity(nc, ident)
            id64 = ident[0:K, 0:K]
            ones_row = sp.tile([1, K], F32)
            nc.vector.memset(ones_row, 1.0)
            ones_col = sp.tile([K, 1], F32)
            nc.vector.memset(ones_col, 1.0)

            if shard:
                # tiny warm-up collective: wakes the CC firmware path early so
                # the real AllReduce doesn't pay the cold-start latency
                warm_s = sp.tile([1, K], F32)
                nc.vector.memset(warm_s, 0.0)
                nc.sync.dma_start(ccw_in[:, :], warm_s)
                nc.gpsimd.collective_compute(
                    "AllReduce",
                    mybir.AluOpType.add,
                    replica_groups=[list(range(N_CORES))],
                    ins=[ccw_in[:, :]],
                    outs=[ccw_out[:, :]],
                )

            # ---------------- projections: AT = fx^T pxT, ByT = fy^T pyT ----
            with tc.tile_pool(name="pacc", bufs=1, space="PSUM") as pacc:
                at_p = pacc.tile([C, K], F32)    # A^T partial  [C,K]
                byt_p = pacc.tile([C, K], F32)   # By^T partial [C,K]
                for n in range(n_chunks):
                    nc.tensor.matmul(at_p, fx_t[:, n, :], pxT_t[:, n, :],
                                     start=(n == 0), stop=(n == n_chunks - 1))
                for n in range(n_chunks):
                    nc.tensor.matmul(byt_p, fy_t[:, n, :], pyT_t[:, n, :],
                                     start=(n == 0), stop=(n == n_chunks - 1))

                if shard:
                    part_s = sp.tile([C, 2 * K], F32)
                    nc.vector.tensor_copy(part_s[:, 0:K], at_p)
                    nc.vector.tensor_copy(part_s[:, K:2 * K], byt_p)
                    nc.sync.dma_start(cc_in[:, :], part_s)
                    nc.gpsimd.collective_compute(
                        "AllReduce",
                        mybir.AluOpType.add,
                        replica_groups=[list(range(N_CORES))],
                        ins=[cc_in[:, :]],
                        outs=[cc_out[:, :]],
                    )
                else:
                    at_s = sb_copy(at_p, [C, K], sp, "at_s")
                    byt_s = sb_copy(byt_p, [C, K], sp, "byt_s")

            # ------- collective-independent work first (hides CC latency) ---
            # G = My^T My
            g_p = ps_tile([K, K])
            nc.tensor.matmul(g_p, my_s, my_s)
            g_s = sb_copy(g_p, [K, K], sp, "g_s")

            # resolvent masks: ev = [ex | ey]; t = ev/max(ev); im = 1/(1+t);
            # re = sqrt(t)*im; both scaled by sqrt(LMBDA)
            evmax = sp.tile([1, 1], F32)
            nc.vector.tensor_reduce(evmax, ev_t, mybir.AxisListType.X,
                                    mybir.AluOpType.max)
            evrec = sp.tile([1, 1], F32)
            nc.vector.reciprocal(evrec, evmax)
            t_t = sp.tile([1, 2 * K], F32)
            nc.vector.tensor_scalar_mul(t_t, ev_t, evrec)
            tp1 = sp.tile([1, 2 * K], F32)
            nc.vector.tensor_scalar_add(tp1, t_t, 1.0)
            im_t = sp.tile([1, 2 * K], F32)
            nc.vector.reciprocal(im_t, tp1)
            sq_t = sp.tile([1, 2 * K], F32)
            nc.scalar.sqrt(sq_t, t_t)
            re_t = sp.tile([1, 2 * K], F32)
            nc.vector.tensor_mul(re_t, sq_t, im_t)
            nc.vector.tensor_scalar_mul(re_t, re_t, SQRT_LMBDA)
            nc.vector.tensor_scalar_mul(im_t, im_t, SQRT_LMBDA)

            # D1T[a,i] = re2[a] - re1[i]; D2T likewise from im
            d_s = []
            for idx, src in enumerate((re_t, im_t)):
                pa = ps_tile([K, K])
                nc.tensor.matmul(pa, src[0:1, K:2 * K], ones_row)  # v2[p]
                pb = ps_tile([K, K])
                nc.tensor.matmul(pb, ones_row, src[0:1, 0:K])      # v1[f]
                ta = sb_copy(pa, [K, K], sp, f"dta{idx}")
                dt = sp.tile([K, K], F32, tag=f"d{idx}t_s", name=f"d{idx}t_s")
                nc.vector.tensor_sub(dt, ta, pb)
                d_s.append(dt)
            d1t_s, d2t_s = d_s

            # Newton-Schulz inverse (S symmetric PD): X' = 2X - X S X
            def newton_inverse(mat_s, tag, steps):
                rs = sp.tile([K, 1], F32, tag=f"{tag}_rs", name=f"{tag}_rs")
                nc.vector.tensor_reduce(rs, mat_s, mybir.AxisListType.X,
                                        mybir.AluOpType.add,
                                        apply_absolute_value=True)
                nc.gpsimd.partition_all_reduce(rs, rs, K, ReduceOp.max)
                al = sp.tile([K, 1], F32, tag=f"{tag}_al", name=f"{tag}_al")
                nc.vector.reciprocal(al, rs)
                x_s = sp.tile([K, K], F32, tag=f"{tag}_x0", name=f"{tag}_x0")
                nc.vector.tensor_scalar_mul(x_s, id64, al)
                for it in range(steps):
                    t1 = ps_tile([K, K])
                    nc.tensor.matmul(t1, mat_s, x_s)          # S X (S sym)
                    t1s = wp.tile([K, K], F32, tag=f"{tag}_t1s",
                                  name=f"{tag}_t1s")
                    nc.vector.tensor_copy(t1s, t1)
                    t2 = ps_tile([K, K])
                    nc.tensor.matmul(t2, x_s, t1s)            # X (S X) (X sym)
                    xn = sp.tile([K, K], F32, tag=f"{tag}_x{it + 1}",
                                 name=f"{tag}_x{it + 1}")
                    nc.vector.scalar_tensor_tensor(
                        xn, x_s, 2.0, t2,
                        op0=mybir.AluOpType.mult,
                        op1=mybir.AluOpType.subtract)
                    x_s = xn
                return x_s

            gi_s = newton_inverse(g_s, "gi", NEWTON_STEPS_G)

            # ------- collective-dependent chain -----------------------------
            if shard:
                full_s = sp.tile([C, 2 * K], F32)
                nc.sync.dma_start(full_s, cc_out[:, :])
                at_s = full_s[:, 0:K]
                byt_s = full_s[:, K:2 * K]

            # S~ = Mx^T (A A^T) Mx    [S_A symmetric -> no transpose]
            sa_p = ps_tile([K, K])
            nc.tensor.matmul(sa_p, at_s, at_s)          # A A^T
            sa_s = sb_copy(sa_p, [K, K], sp, "sa_s")
            h1t_p = ps_tile([K, K])
            nc.tensor.matmul(h1t_p, sa_s, mx_s)         # S_A Mx (sym trick)
            h1t_s = sb_copy(h1t_p, [K, K], sp, "h1t_s")
            st_p = ps_tile([K, K])
            nc.tensor.matmul(st_p, mx_s, h1t_s)         # Mx^T S_A Mx
            st_s = sb_copy(st_p, [K, K], sp, "st_s")

            # RHS' = My^T Bc A^T Mx = My^T (My (By A^T)) Mx
            byat_p = ps_tile([K, K])
            nc.tensor.matmul(byat_p, byt_s, at_s)       # By A^T
            byat_s = sb_copy(byat_p, [K, K], sp, "byat_s")
            bca_p = ps_tile([K, K])
            nc.tensor.matmul(bca_p, myT_s, byat_s)      # My (By A^T) = Bc A^T
            bca_s = sb_copy(bca_p, [K, K], sp, "bca_s")
            w_p = ps_tile([K, K])
            nc.tensor.matmul(w_p, my_s, bca_s)          # My^T Bc A^T
            w_s = sb_copy(w_p, [K, K], sp, "w_s")
            wt_p = ps_tile([K, K])
            nc.tensor.transpose(wt_p, w_s, id64)
            wt_s = sb_copy(wt_p, [K, K], sp, "wt_s")
            rp_p = ps_tile([K, K])
            nc.tensor.matmul(rp_p, wt_s, mx_s)          # (My^T Bc A^T) Mx
            r_s = sp.tile([K, K], F32)                  # CG residual
            nc.vector.tensor_copy(r_s, rp_p)

            si_s = newton_inverse(st_s, "si", NEWTON_STEPS_S)

            # ---------------- PCG (transposed space, transpose-free) --------
            y_s = sp.tile([K, K], F32)
            nc.vector.memset(y_s, 0.0)
            p_s = sp.tile([K, K], F32)
            u_s = sp.tile([K, 2 * K], F32)   # stacked [D1T*p | D2T*p]

            def precond(r_tile):
                """z = (Gi r) Si via (Gi r)^T = mm(lhsT=r, rhs=Gi)."""
                ut_p = ps_tile([K, K])
                nc.tensor.matmul(ut_p, r_tile, gi_s)          # (Gi r)^T
                ut_s = wp.tile([K, K], F32, tag="pc_uts", name="pc_uts")
                nc.vector.tensor_copy(ut_s, ut_p)
                z_p = ps_tile([K, K])
                nc.tensor.matmul(z_p, ut_s, si_s)             # (Gi r) Si
                return z_p

            def dot_b(a_ap, b_ap, tag):
                """<a,b> broadcast to all partitions as [K,1] SBUF."""
                prod = wp.tile([K, K], F32, tag="dot_dm", name="dot_dm")
                acc = wp.tile([K, 1], F32, tag=f"{tag}_acc", name=f"{tag}_acc")
                nc.vector.scalar_tensor_tensor(
                    prod, a_ap, 1.0, b_ap,
                    op0=mybir.AluOpType.bypass, op1=mybir.AluOpType.mult,
                    accum_out=acc)
                nc.gpsimd.partition_all_reduce(acc, acc, K, ReduceOp.add)
                return acc

            z_p = precond(r_s)
            nc.vector.tensor_copy(p_s, z_p)
            rz0 = dot_b(r_s, z_p, "rz")
            rzrec = wp.tile([K, 1], F32, tag="rzrec", name="rzrec")
            nc.vector.reciprocal(rzrec, rz0)

            for it in range(N_ITERS):
                # ---- q = (G p) S~ + sum_d DdT*(G(DdT*p)) ----
                nc.vector.tensor_mul(u_s[:, 0:K], d1t_s, p_s)
                nc.vector.tensor_mul(u_s[:, K:2 * K], d2t_s, p_s)
                gpt_p = ps_tile([K, K])
                nc.tensor.matmul(gpt_p, p_s, g_s)         # (G p)^T
                gpt_s = wp.tile([K, K], F32, tag="mv_gpts", name="mv_gpts")
                nc.vector.tensor_copy(gpt_s, gpt_p)
                t2_p = ps_tile([K, K])
                nc.tensor.matmul(t2_p, gpt_s, st_s)       # (G p) S~
                gu_p = ps_tile([K, 2 * K])
                nc.tensor.matmul(gu_p, g_s, u_s)          # [G u1 | G u2]
                m1_s = wp.tile([K, K], F32, tag="mv_m1", name="mv_m1")
                nc.vector.tensor_mul(m1_s, d1t_s, gu_p[:, 0:K])
                m2_s = wp.tile([K, K], F32, tag="mv_m2", name="mv_m2")
                nc.vector.tensor_mul(m2_s, d2t_s, gu_p[:, K:2 * K])
                ms_s = wp.tile([K, K], F32, tag="mv_ms", name="mv_ms")
                nc.vector.tensor_add(ms_s, m1_s, m2_s)
                q_s = wp.tile([K, K], F32, tag="mv_q", name="mv_q")
                nc.vector.tensor_add(q_s, ms_s, t2_p)

                # ---- alpha = rz/<p,q>; y += alpha p; r -= alpha q ----
                pq = dot_b(p_s, q_s, "pq")
                pqr = wp.tile([K, 1], F32, tag="pqr", name="pqr")
                nc.vector.reciprocal(pqr, pq)
                al = wp.tile([K, 1], F32, tag="al", name="al")
                nc.vector.tensor_mul(al, rz0, pqr)
                an = wp.tile([K, 1], F32, tag="an", name="an")
                nc.vector.tensor_scalar_mul(an, al, -1.0)
                nc.vector.scalar_tensor_tensor(
                    y_s, p_s, al, y_s,
                    op0=mybir.AluOpType.mult, op1=mybir.AluOpType.add)
                nc.vector.scalar_tensor_tensor(
                    r_s, q_s, an, r_s,
                    op0=mybir.AluOpType.mult, op1=mybir.AluOpType.add)

                if it == N_ITERS - 1:
                    break

                # ---- z = P^-1 r; beta = <r,z>/rz; p = beta p + z ----
                z_p = precond(r_s)
                rz_new = dot_b(r_s, z_p, "rz")
                bt = wp.tile([K, 1], F32, tag="bt", name="bt")
                nc.vector.tensor_mul(bt, rz_new, rzrec)
                nc.vector.scalar_tensor_tensor(
                    p_s, p_s, bt, z_p,
                    op0=mybir.AluOpType.mult, op1=mybir.AluOpType.add)
                rz0 = rz_new
                rzrec = wp.tile([K, 1], F32, tag="rzrec", name="rzrec")
                nc.vector.reciprocal(rzrec, rz0)

            # ---------------- output: C = Y Mx^T ----------------
            yt_p = ps_tile([K, K])
            nc.tensor.transpose(yt_p, y_s, id64)
            yt_s = wp.tile([K, K], F32, tag="yt_s", name="yt_s")
            nc.vector.tensor_copy(yt_s, yt_p)
            c_p = ps_tile([K, K])
            nc.tensor.matmul(c_p, yt_s, mxT_s)      # Y Mx^T
            c_s = wp.tile([K, K], F32, tag="c_s", name="c_s")
            nc.vector.tensor_copy(c_s, c_p)
            nc.sync.dma_start(out_d[:, :], c_s)

    nc.finalize()
    return nc


def get_program(shard: bool):
    if shard not in _PROGRAM_CACHE:
        _PROGRAM_CACHE[shard] = build_program(shard)
    return _PROGRAM_CACHE[shard]


def make_in_maps(inputs, shard: bool):
    fx = np.ascontiguousarray(np.asarray(inputs["feat_x"], np.float32)[0])
    fy = np.ascontiguousarray(np.asarray(inputs["feat_y"], np.float32)[0])
    pxT = np.ascontiguousarray(np.asarray(inputs["evecs_trans_x"], np.float32)[0].T)
    pyT = np.ascontiguousarray(np.asarray(inputs["evecs_trans_y"], np.float32)[0].T)
    mx = np.ascontiguousarray(np.asarray(inputs["sqrtMk_x"], np.float32)[0])
    my = np.ascontiguousarray(np.asarray(inputs["sqrtMk_y"], np.float32)[0])
    ev = np.ascontiguousarray(np.concatenate([
        np.asarray(inputs["evals_x"], np.float32)[0],
        np.asarray(inputs["evals_y"], np.float32)[0],
    ])[None, :])
    small = {
        "mx": mx, "my": my,
        "mxT": np.ascontiguousarray(mx.T),
        "myT": np.ascontiguousarray(my.T),
        "ev": ev,
    }
    in_maps = []
    for c in range(N_CORES):
        if shard:
            lo, hi = c * (V // N_CORES), (c + 1) * (V // N_CORES)
            m = {"fx": fx[lo:hi], "fy": fy[lo:hi],
                 "pxT": pxT[lo:hi], "pyT": pyT[lo:hi]}
        else:
            m = {"fx": fx, "fy": fy, "pxT": pxT, "pyT": pyT}
        m.update(small)
        in_maps.append(m)
    return in_maps


def kernel(**inputs) -> np.ndarray:
    nc = get_program(SHARD)
    in_maps = make_in_maps(inputs, SHARD)
    res = run_bass_kernel_spmd(nc, in_maps, core_ids=list(range(N_CORES)))
    out = np.asarray(res.results[0]["out"], dtype=np.float32)
    return out[None]


# revision 16
# speedup vs baseline: 1.6617x; 1.0725x over previous
"""Trainium2 Bass kernel for nn_ExpandedResolventFMNet.

Mathematical reformulation (validated in fp64 against the jax reference):

The reference builds kron(A.T, My) [8192x4096], its Gram [4096^2], resolvent
kron masks, and solves a dense 4096x4096 system.  All of that collapses:

  first        = kron(A A^T, G),              G = My^T My
  second       = kron-sum of 64x64 factors; with X = Mx W the full system is
  M(W)         = S~ W G + LMBDA * sum_d Dd*( (Dd*W) G ) = R~    (* = Hadamard)
  S~           = Mx^T (A A^T) Mx
  R~           = Mx^T A Bc^T My
  Dd           = resolvent-mask difference matrices (64x64)
  output C     = (Mx W)^T

The 4096x4096 operator kron(S~,G)+LMBDA*blockdiag is SPD with cond ~3e2; PCG
with the exact-kron preconditioner P^-1 = kron(S~^-1, G^-1) (applied as two
64x64 matmuls) converges to the fp32 floor in <=14 iterations.  The device
runs the transposed system in Y = W^T:

  M'(Y) = G Y S~ + sum_d DdT * (G (DdT * Y)),   C = Y Mx^T

and exploits symmetry so that every matmul is transpose-free:
  (G p)^T   = mm(lhsT=p,  rhs=G)     [G symmetric]
  (G p) S~  = mm(lhsT=(G p)^T, rhs=S~)
  (Gi r)^T  = mm(lhsT=r,  rhs=Gi)
  (Gi r) Si = mm(lhsT=(Gi r)^T, rhs=Si)

S~^-1 and G^-1 are produced on-device by Newton-Schulz iteration.
sqrt(LMBDA) is folded into DdT.  Work is sharded over 8 cores for the
V=5000 projections (AllReduce of the 64KB partials); the small solve runs
redundantly on every core.
"""

import numpy as np

import concourse.bacc as bacc
import concourse.mybir as mybir
from concourse.bass_isa import ReduceOp
from concourse.bass_utils import run_bass_kernel_spmd
from concourse.masks import make_identity
from concourse.tile import TileContext

F32 = mybir.dt.float32
K = 64          # spectral basis size
C = 128         # feature channels
V = 5000        # vertices
CHUNK = 125     # v-contraction tile (partition dim)
N_CORES = 8
N_ITERS = 14
NEWTON_STEPS_S = 8
NEWTON_STEPS_G = 4
SQRT_LMBDA = 10.0

SHARD = True    # shard projections over cores + AllReduce partials

_PROGRAM_CACHE = {}


def build_program(shard: bool):
    nc = bacc.Bacc("TRN2", num_devices=N_CORES)
    v_local = V // N_CORES if shard else V          # 625 or 5000
    n_chunks = v_local // CHUNK                     # 5 or 40

    fx_d = nc.dram_tensor("fx", [v_local, C], F32, kind="ExternalInput")
    fy_d = nc.dram_tensor("fy", [v_local, C], F32, kind="ExternalInput")
    pxT_d = nc.dram_tensor("pxT", [v_local, K], F32, kind="ExternalInput")
    pyT_d = nc.dram_tensor("pyT", [v_local, K], F32, kind="ExternalInput")
    mx_d = nc.dram_tensor("mx", [K, K], F32, kind="ExternalInput")
    my_d = nc.dram_tensor("my", [K, K], F32, kind="ExternalInput")
    mxT_d = nc.dram_tensor("mxT", [K, K], F32, kind="ExternalInput")
    myT_d = nc.dram_tensor("myT", [K, K], F32, kind="ExternalInput")
    ev_d = nc.dram_tensor("ev", [1, 2 * K], F32, kind="ExternalInput")
    out_d = nc.dram_tensor("out", [K, K], F32, kind="ExternalOutput")

    if shard:
        cc_in = nc.dram_tensor("cc_in", [C, 2 * K], F32)
        cc_out = nc.dram_tensor("cc_out", [C, 2 * K], F32, addr_space="Shared")
        ccw_in = nc.dram_tensor("ccw_in", [1, K], F32)
        ccw_out = nc.dram_tensor("ccw_out", [1, K], F32, addr_space="Shared")

    with TileContext(nc) as tc:
        with (
            tc.tile_pool(name="big", bufs=1) as bp,
            tc.tile_pool(name="persist", bufs=1) as sp,
            tc.tile_pool(name="work", bufs=2) as wp,
            tc.tile_pool(name="psum", bufs=2, space="PSUM") as pp,
        ):
            if shard:
                # tiny warm-up collective at t=0 (gpsimd DMA queue, which is
                # otherwise idle) to absorb the CC firmware cold-start before
                # the real AllReduce arrives
                warm_s = sp.tile([1, K], F32)
                nc.vector.memset(warm_s, 0.0)
                nc.gpsimd.dma_start(ccw_in[:, :], warm_s)
                nc.gpsimd.collective_compute(
                    "AllReduce",
                    mybir.AluOpType.add,
                    replica_groups=[list(range(N_CORES))],
                    ins=[ccw_in[:, :]],
                    outs=[ccw_out[:, :]],
                )

            # rotating psum tags: 3 tags x bufs=2 -> 6 banks (+proj acc 2 = 8)
            _ps_state = {"i": 0}

            def ps_tile(shape):
                i = _ps_state["i"]
                _ps_state["i"] += 1
                return pp.tile(shape, F32, tag=f"ps{i % 3}", name=f"pst{i}")

            def sb_copy(src_psum, shape, pool, tag, engine="vector"):
                t = pool.tile(shape, F32, tag=tag, name=tag)
                if engine == "vector":
                    nc.vector.tensor_copy(t, src_psum)
                else:
                    nc.scalar.copy(t, src_psum)
                return t

            # ---------------- input DMA (one DMA per big tensor) ------------
            fx_t = bp.tile([CHUNK, n_chunks, C], F32)
            fy_t = bp.tile([CHUNK, n_chunks, C], F32)
            pxT_t = bp.tile([CHUNK, n_chunks, K], F32)
            pyT_t = bp.tile([CHUNK, n_chunks, K], F32)
            nc.sync.dma_start(
                fx_t, fx_d.rearrange("(n p) c -> p n c", p=CHUNK))
            nc.sync.dma_start(
                fy_t, fy_d.rearrange("(n p) c -> p n c", p=CHUNK))
            nc.sync.dma_start(
                pxT_t, pxT_d.rearrange("(n p) c -> p n c", p=CHUNK))
            nc.sync.dma_start(
                pyT_t, pyT_d.rearrange("(n p) c -> p n c", p=CHUNK))
            mx_s = sp.tile([K, K], F32)
            my_s = sp.tile([K, K], F32)
            mxT_s = sp.tile([K, K], F32)
            myT_s = sp.tile([K, K], F32)
            ev_t = sp.tile([1, 2 * K], F32)
            nc.sync.dma_start(mx_s, mx_d[:, :])
            nc.sync.dma_start(my_s, my_d[:, :])
            nc.sync.dma_start(mxT_s, mxT_d[:, :])
            nc.sync.dma_start(myT_s, myT_d[:, :])
            nc.sync.dma_start(ev_t, ev_d[:, :])

            ident = sp.tile([C, C], F32)
            make_identity(nc, ident)
            id64 = ident[0:K, 0:K]
            ones_row = sp.tile([1, K], F32)
            nc.vector.memset(ones_row, 1.0)
            ones_col = sp.tile([K, 1], F32)
            nc.vector.memset(ones_col, 1.0)


            # ---------------- projections: AT = fx^T pxT, ByT = fy^T pyT ----
            with tc.tile_pool(name="pacc", bufs=1, space="PSUM") as pacc:
                at_p = pacc.tile([C, K], F32)    # A^T partial  [C,K]
                byt_p = pacc.tile([C, K], F32)   # By^T partial [C,K]
                for n in range(n_chunks):
                    nc.tensor.matmul(at_p, fx_t[:, n, :], pxT_t[:, n, :],
                                     start=(n == 0), stop=(n == n_chunks - 1))
                for n in range(n_chunks):
                    nc.tensor.matmul(byt_p, fy_t[:, n, :], pyT_t[:, n, :],
                                     start=(n == 0), stop=(n == n_chunks - 1))

                if shard:
                    part_s = sp.tile([C, 2 * K], F32)
                    nc.vector.tensor_copy(part_s[:, 0:K], at_p)
                    nc.vector.tensor_copy(part_s[:, K:2 * K], byt_p)
                    nc.sync.dma_start(cc_in[:, :], part_s)
                    nc.gpsimd.collective_compute(
                        "AllReduce",
                        mybir.AluOpType.add,
                        replica_groups=[list(range(N_CORES))],
                        ins=[cc_in[:, :]],
                        outs=[cc_out[:, :]],
                    )
                else:
                    at_s = sb_copy(at_p, [C, K], sp, "at_s")
                    byt_s = sb_copy(byt_p, [C, K], sp, "byt_s")

            # ------- collective-independent work first (hides CC latency) ---
            # G = My^T My
            g_p = ps_tile([K, K])
            nc.tensor.matmul(g_p, my_s, my_s)
            g_s = sb_copy(g_p, [K, K], sp, "g_s")

            # resolvent masks: ev = [ex | ey]; t = ev/max(ev); im = 1/(1+t);
            # re = sqrt(t)*im; both scaled by sqrt(LMBDA)
            evmax = sp.tile([1, 1], F32)
            nc.vector.tensor_reduce(evmax, ev_t, mybir.AxisListType.X,
                                    mybir.AluOpType.max)
            evrec = sp.tile([1, 1], F32)
            nc.vector.reciprocal(evrec, evmax)
            t_t = sp.tile([1, 2 * K], F32)
            nc.vector.tensor_scalar_mul(t_t, ev_t, evrec)
            tp1 = sp.tile([1, 2 * K], F32)
            nc.vector.tensor_scalar_add(tp1, t_t, 1.0)
            im_t = sp.tile([1, 2 * K], F32)
            nc.vector.reciprocal(im_t, tp1)
            sq_t = sp.tile([1, 2 * K], F32)
            nc.scalar.sqrt(sq_t, t_t)
            re_t = sp.tile([1, 2 * K], F32)
            nc.vector.tensor_mul(re_t, sq_t, im_t)
            nc.vector.tensor_scalar_mul(re_t, re_t, SQRT_LMBDA)
            nc.vector.tensor_scalar_mul(im_t, im_t, SQRT_LMBDA)

            # D1T[a,i] = re2[a] - re1[i]; D2T likewise from im
            d_s = []
            for idx, src in enumerate((re_t, im_t)):
                pa = ps_tile([K, K])
                nc.tensor.matmul(pa, src[0:1, K:2 * K], ones_row)  # v2[p]
                pb = ps_tile([K, K])
                nc.tensor.matmul(pb, ones_row, src[0:1, 0:K])      # v1[f]
                ta = sb_copy(pa, [K, K], sp, f"dta{idx}")
                dt = sp.tile([K, K], F32, tag=f"d{idx}t_s", name=f"d{idx}t_s")
                nc.vector.tensor_sub(dt, ta, pb)
                d_s.append(dt)
            d1t_s, d2t_s = d_s

            # Newton-Schulz inverse (S symmetric PD): X' = 2X - X S X
            def newton_inverse(mat_s, tag, steps):
                rs = sp.tile([K, 1], F32, tag=f"{tag}_rs", name=f"{tag}_rs")
                nc.vector.tensor_reduce(rs, mat_s, mybir.AxisListType.X,
                                        mybir.AluOpType.add,
                                        apply_absolute_value=True)
                nc.gpsimd.partition_all_reduce(rs, rs, K, ReduceOp.max)
                al = sp.tile([K, 1], F32, tag=f"{tag}_al", name=f"{tag}_al")
                nc.vector.reciprocal(al, rs)
                x_s = sp.tile([K, K], F32, tag=f"{tag}_x0", name=f"{tag}_x0")
                nc.vector.tensor_scalar_mul(x_s, id64, al)
                for it in range(steps):
                    t1 = ps_tile([K, K])
                    nc.tensor.matmul(t1, mat_s, x_s)          # S X (S sym)
                    t1s = wp.tile([K, K], F32, tag=f"{tag}_t1s",
                                  name=f"{tag}_t1s")
                    nc.vector.tensor_copy(t1s, t1)
                    t2 = ps_tile([K, K])
                    nc.tensor.matmul(t2, x_s, t1s)            # X (S X) (X sym)
                    xn = sp.tile([K, K], F32, tag=f"{tag}_x{it + 1}",
                                 name=f"{tag}_x{it + 1}")
                    nc.vector.scalar_tensor_tensor(
                        xn, x_s, 2.0, t2,
                        op0=mybir.AluOpType.mult,
                        op1=mybir.AluOpType.subtract)
                    x_s = xn
                return x_s

            gi_s = newton_inverse(g_s, "gi", NEWTON_STEPS_G)

            # ------- collective-dependent chain -----------------------------
            if shard:
                full_s = sp.tile([C, 2 * K], F32)
                nc.sync.dma_start(full_s, cc_out[:, :])
                at_s = full_s[:, 0:K]
                byt_s = full_s[:, K:2 * K]

            # S~ = Mx^T (A A^T) Mx    [S_A symmetric -> no transpose]
            sa_p = ps_tile([K, K])
            nc.tensor.matmul(sa_p, at_s, at_s)          # A A^T
            sa_s = sb_copy(sa_p, [K, K], sp, "sa_s")
            h1t_p = ps_tile([K, K])
            nc.tensor.matmul(h1t_p, sa_s, mx_s)         # S_A Mx (sym trick)
            h1t_s = sb_copy(h1t_p, [K, K], sp, "h1t_s")
            st_p = ps_tile([K, K])
            nc.tensor.matmul(st_p, mx_s, h1t_s)         # Mx^T S_A Mx
            st_s = sb_copy(st_p, [K, K], sp, "st_s")

            # RHS' = My^T Bc A^T Mx = My^T (My (By A^T)) Mx
            byat_p = ps_tile([K, K])
            nc.tensor.matmul(byat_p, byt_s, at_s)       # By A^T
            byat_s = sb_copy(byat_p, [K, K], sp, "byat_s")
            bca_p = ps_tile([K, K])
            nc.tensor.matmul(bca_p, myT_s, byat_s)      # My (By A^T) = Bc A^T
            bca_s = sb_copy(bca_p, [K, K], sp, "bca_s")
            w_p = ps_tile([K, K])
            nc.tensor.matmul(w_p, my_s, bca_s)          # My^T Bc A^T
            w_s = sb_copy(w_p, [K, K], sp, "w_s")
            wt_p = ps_tile([K, K])
            nc.tensor.transpose(wt_p, w_s, id64)
            wt_s = sb_copy(wt_p, [K, K], sp, "wt_s")
            rp_p = ps_tile([K, K])
            nc.tensor.matmul(rp_p, wt_s, mx_s)          # (My^T Bc A^T) Mx
            r_s = sp.tile([K, K], F32)                  # CG residual
            nc.vector.tensor_copy(r_s, rp_p)

            si_s = newton_inverse(st_s, "si", NEWTON_STEPS_S)

            # ---------------- PCG (transposed space, transpose-free) --------
            y_s = sp.tile([K, K], F32)
            nc.vector.memset(y_s, 0.0)
            p_s = sp.tile([K, K], F32)
            u_s = sp.tile([K, 2 * K], F32)   # stacked [D1T*p | D2T*p]

            def precond(r_tile):
                """z = (Gi r) Si via (Gi r)^T = mm(lhsT=r, rhs=Gi)."""
                ut_p = ps_tile([K, K])
                nc.tensor.matmul(ut_p, r_tile, gi_s)          # (Gi r)^T
                ut_s = wp.tile([K, K], F32, tag="pc_uts", name="pc_uts")
                nc.vector.tensor_copy(ut_s, ut_p)
                z_p = ps_tile([K, K])
                nc.tensor.matmul(z_p, ut_s, si_s)             # (Gi r) Si
                return z_p

            def dot_b(a_ap, b_ap, tag):
                """<a,b> broadcast to all partitions as [K,1] SBUF."""
                prod = wp.tile([K, K], F32, tag="dot_dm", name="dot_dm")
                acc = wp.tile([K, 1], F32, tag=f"{tag}_acc", name=f"{tag}_acc")
                nc.vector.scalar_tensor_tensor(
                    prod, a_ap, 1.0, b_ap,
                    op0=mybir.AluOpType.bypass, op1=mybir.AluOpType.mult,
                    accum_out=acc)
                nc.gpsimd.partition_all_reduce(acc, acc, K, ReduceOp.add)
                return acc

            z_p = precond(r_s)
            z_s = sp.tile([K, K], F32)
            nc.vector.tensor_copy(z_s, z_p)
            nc.vector.tensor_copy(p_s, z_p)
            rz0 = dot_b(r_s, z_p, "rz")
            rzrec = wp.tile([K, 1], F32, tag="rzrec", name="rzrec")
            nc.vector.reciprocal(rzrec, rz0)
            rzneg = wp.tile([K, 1], F32, tag="rzneg", name="rzneg")
            nc.vector.tensor_scalar_mul(rzneg, rz0, -1.0)

            for it in range(N_ITERS):
                # ---- q = (G p) S~ + sum_d DdT*(G(DdT*p)) ----
                nc.vector.tensor_mul(u_s[:, 0:K], d1t_s, p_s)
                nc.vector.tensor_mul(u_s[:, K:2 * K], d2t_s, p_s)
                gpt_p = ps_tile([K, K])
                nc.tensor.matmul(gpt_p, p_s, g_s)         # (G p)^T
                gpt_s = wp.tile([K, K], F32, tag="mv_gpts", name="mv_gpts")
                nc.vector.tensor_copy(gpt_s, gpt_p)
                t2_p = ps_tile([K, K])
                nc.tensor.matmul(t2_p, gpt_s, st_s)       # (G p) S~
                gu_p = ps_tile([K, 2 * K])
                nc.tensor.matmul(gu_p, g_s, u_s)          # [G u1 | G u2]
                m1_s = wp.tile([K, K], F32, tag="mv_m1", name="mv_m1")
                nc.vector.tensor_mul(m1_s, d1t_s, gu_p[:, 0:K])
                m2_s = wp.tile([K, K], F32, tag="mv_m2", name="mv_m2")
                nc.vector.tensor_mul(m2_s, d2t_s, gu_p[:, K:2 * K])
                ms_s = wp.tile([K, K], F32, tag="mv_ms", name="mv_ms")
                nc.vector.tensor_add(ms_s, m1_s, m2_s)
                q_s = wp.tile([K, K], F32, tag="mv_q", name="mv_q")
                nc.vector.tensor_add(q_s, ms_s, t2_p)

                # ---- alpha = rz/<p,q>; y += alpha p; r -= alpha q ----
                # zq = P^-1 q runs on PE concurrently with the <p,q> dot, so
                # the preconditioner is off the critical path (z, rz still
                # exact: z updated by recurrence z -= alpha zq, rz by dot).
                pq = dot_b(p_s, q_s, "pq")
                if it < N_ITERS - 1:
                    zq_p = precond(q_s)
                pqr = wp.tile([K, 1], F32, tag="pqr", name="pqr")
                nc.vector.reciprocal(pqr, pq)
                al = wp.tile([K, 1], F32, tag="al", name="al")
                nc.vector.tensor_mul(al, rz0, pqr)
                an = wp.tile([K, 1], F32, tag="an", name="an")
                nc.vector.tensor_mul(an, rzneg, pqr)
                nc.vector.scalar_tensor_tensor(
                    y_s, p_s, al, y_s,
                    op0=mybir.AluOpType.mult, op1=mybir.AluOpType.add)
                nc.vector.scalar_tensor_tensor(
                    r_s, q_s, an, r_s,
                    op0=mybir.AluOpType.mult, op1=mybir.AluOpType.add)

                if it == N_ITERS - 1:
                    break

                # ---- z -= alpha zq; beta = <r,z>/rz; p = beta p + z ----
                nc.vector.scalar_tensor_tensor(
                    z_s, zq_p, an, z_s,
                    op0=mybir.AluOpType.mult, op1=mybir.AluOpType.add)
                rz_new = dot_b(r_s, z_s, "rz")
                bt = wp.tile([K, 1], F32, tag="bt", name="bt")
                nc.vector.tensor_mul(bt, rz_new, rzrec)
                nc.vector.scalar_tensor_tensor(
                    p_s, p_s, bt, z_s,
                    op0=mybir.AluOpType.mult, op1=mybir.AluOpType.add)
                rz0 = rz_new
                rzrec = wp.tile([K, 1], F32, tag="rzrec", name="rzrec")
                nc.vector.reciprocal(rzrec, rz0)
                rzneg = wp.tile([K, 1], F32, tag="rzneg", name="rzneg")
                nc.vector.tensor_scalar_mul(rzneg, rz0, -1.0)

            # ---------------- output: C = Y Mx^T ----------------
            yt_p = ps_tile([K, K])
            nc.tensor.transpose(yt_p, y_s, id64)
            yt_s = wp.tile([K, K], F32, tag="yt_s", name="yt_s")
            nc.vector.tensor_copy(yt_s, yt_p)
            c_p = ps_tile([K, K])
            nc.tensor.matmul(c_p, yt_s, mxT_s)      # Y Mx^T
            c_s = wp.tile([K, K], F32, tag="c_s", name="c_s")
            nc.vector.tensor_copy(c_s, c_p)
            nc.sync.dma_start(out_d[:, :], c_s)

    nc.finalize()
    return nc


def get_program(shard: bool):
    if shard not in _PROGRAM_CACHE:
        _PROGRAM_CACHE[shard] = build_program(shard)
    return _PROGRAM_CACHE[shard]


def make_in_maps(inputs, shard: bool):
    fx = np.ascontiguousarray(np.asarray(inputs["feat_x"], np.float32)[0])
    fy = np.ascontiguousarray(np.asarray(inputs["feat_y"], np.float32)[0])
    pxT = np.ascontiguousarray(np.asarray(inputs["evecs_trans_x"], np.float32)[0].T)
    pyT = np.ascontiguousarray(np.asarray(inputs["evecs_trans_y"], np.float32)[0].T)
    mx = np.ascontiguousarray(np.asarray(inputs["sqrtMk_x"], np.float32)[0])
    my = np.ascontiguousarray(np.asarray(inputs["sqrtMk_y"], np.float32)[0])
    ev = np.ascontiguousarray(np.concatenate([
        np.asarray(inputs["evals_x"], np.float32)[0],
        np.asarray(inputs["evals_y"], np.float32)[0],
    ])[None, :])
    small = {
        "mx": mx, "my": my,
        "mxT": np.ascontiguousarray(mx.T),
        "myT": np.ascontiguousarray(my.T),
        "ev": ev,
    }
    in_maps = []
    for c in range(N_CORES):
        if shard:
            lo, hi = c * (V // N_CORES), (c + 1) * (V // N_CORES)
            m = {"fx": fx[lo:hi], "fy": fy[lo:hi],
                 "pxT": pxT[lo:hi], "pyT": pyT[lo:hi]}
        else:
            m = {"fx": fx, "fy": fy, "pxT": pxT, "pyT": pyT}
        m.update(small)
        in_maps.append(m)
    return in_maps


def kernel(**inputs) -> np.ndarray:
    nc = get_program(SHARD)
    in_maps = make_in_maps(inputs, SHARD)
    res = run_bass_kernel_spmd(nc, in_maps, core_ids=list(range(N_CORES)))
    out = np.asarray(res.results[0]["out"], dtype=np.float32)
    return out[None]


# revision 17
# speedup vs baseline: 2.1298x; 1.2817x over previous
"""Trainium2 Bass kernel for nn_ExpandedResolventFMNet.

Mathematical reformulation (validated in fp64 against the jax reference):

The reference builds kron(A.T, My) [8192x4096], its Gram [4096^2], resolvent
kron masks, and solves a dense 4096x4096 system.  All of that collapses:

  first        = kron(A A^T, G),              G = My^T My
  second       = kron-sum of 64x64 factors; with X = Mx W the full system is
  M(W)         = S~ W G + LMBDA * sum_d Dd*( (Dd*W) G ) = R~    (* = Hadamard)
  S~           = Mx^T (A A^T) Mx
  R~           = Mx^T A Bc^T My
  Dd           = resolvent-mask difference matrices (64x64)
  output C     = (Mx W)^T

The 4096x4096 operator kron(S~,G)+LMBDA*blockdiag is SPD with cond ~3e2; PCG
with the exact-kron preconditioner P^-1 = kron(S~^-1, G^-1) (applied as two
64x64 matmuls) converges to the fp32 floor in <=14 iterations.  The device
runs the transposed system in Y = W^T:

  M'(Y) = G Y S~ + sum_d DdT * (G (DdT * Y)),   C = Y Mx^T

and exploits symmetry so that every matmul is transpose-free:
  (G p)^T   = mm(lhsT=p,  rhs=G)     [G symmetric]
  (G p) S~  = mm(lhsT=(G p)^T, rhs=S~)
  (Gi r)^T  = mm(lhsT=r,  rhs=Gi)
  (Gi r) Si = mm(lhsT=(Gi r)^T, rhs=Si)

S~^-1 and G^-1 are produced on-device by Newton-Schulz iteration.
sqrt(LMBDA) is folded into DdT.  Work is sharded over 8 cores for the
V=5000 projections (AllReduce of the 64KB partials); the small solve runs
redundantly on every core.
"""

import numpy as np

import concourse.bacc as bacc
import concourse.mybir as mybir
from concourse.bass_isa import ReduceOp
from concourse.bass_utils import run_bass_kernel_spmd
from concourse.masks import make_identity
from concourse.tile import TileContext

F32 = mybir.dt.float32
K = 64          # spectral basis size
C = 128         # feature channels
V = 5000        # vertices
CHUNK = 125     # v-contraction tile (partition dim)
N_CORES = 8
N_ITERS = 12
NEWTON_STEPS_S = 8
NEWTON_STEPS_G = 4
SQRT_LMBDA = 10.0

SHARD = True    # shard projections over cores + AllReduce partials

_PROGRAM_CACHE = {}


def build_program(shard: bool):
    nc = bacc.Bacc("TRN2", num_devices=N_CORES)
    v_local = V // N_CORES if shard else V          # 625 or 5000
    n_chunks = v_local // CHUNK                     # 5 or 40

    fx_d = nc.dram_tensor("fx", [v_local, C], F32, kind="ExternalInput")
    fy_d = nc.dram_tensor("fy", [v_local, C], F32, kind="ExternalInput")
    pxT_d = nc.dram_tensor("pxT", [v_local, K], F32, kind="ExternalInput")
    pyT_d = nc.dram_tensor("pyT", [v_local, K], F32, kind="ExternalInput")
    mx_d = nc.dram_tensor("mx", [K, K], F32, kind="ExternalInput")
    my_d = nc.dram_tensor("my", [K, K], F32, kind="ExternalInput")
    mxT_d = nc.dram_tensor("mxT", [K, K], F32, kind="ExternalInput")
    myT_d = nc.dram_tensor("myT", [K, K], F32, kind="ExternalInput")
    ev_d = nc.dram_tensor("ev", [1, 2 * K], F32, kind="ExternalInput")
    out_d = nc.dram_tensor("out", [K, K], F32, kind="ExternalOutput")

    if shard:
        cc_in = nc.dram_tensor("cc_in", [C, 2 * K], F32)
        cc_out = nc.dram_tensor("cc_out", [C, 2 * K], F32, addr_space="Shared")

    with TileContext(nc) as tc:
        with (
            tc.tile_pool(name="big", bufs=1) as bp,
            tc.tile_pool(name="persist", bufs=1) as sp,
            tc.tile_pool(name="work", bufs=2) as wp,
            tc.tile_pool(name="psum", bufs=2, space="PSUM") as pp,
        ):

            # rotating psum tags: 3 tags x bufs=2 -> 6 banks (+proj acc 2 = 8)
            _ps_state = {"i": 0}

            def ps_tile(shape):
                i = _ps_state["i"]
                _ps_state["i"] += 1
                return pp.tile(shape, F32, tag=f"ps{i % 3}", name=f"pst{i}")

            def sb_copy(src_psum, shape, pool, tag, engine="vector"):
                t = pool.tile(shape, F32, tag=tag, name=tag)
                if engine == "vector":
                    nc.vector.tensor_copy(t, src_psum)
                else:
                    nc.scalar.copy(t, src_psum)
                return t

            # ---------------- input DMA (one DMA per big tensor) ------------
            fx_t = bp.tile([CHUNK, n_chunks, C], F32)
            fy_t = bp.tile([CHUNK, n_chunks, C], F32)
            pxT_t = bp.tile([CHUNK, n_chunks, K], F32)
            pyT_t = bp.tile([CHUNK, n_chunks, K], F32)
            nc.sync.dma_start(
                fx_t, fx_d.rearrange("(n p) c -> p n c", p=CHUNK))
            nc.sync.dma_start(
                fy_t, fy_d.rearrange("(n p) c -> p n c", p=CHUNK))
            nc.sync.dma_start(
                pxT_t, pxT_d.rearrange("(n p) c -> p n c", p=CHUNK))
            nc.sync.dma_start(
                pyT_t, pyT_d.rearrange("(n p) c -> p n c", p=CHUNK))
            mx_s = sp.tile([K, K], F32)
            my_s = sp.tile([K, K], F32)
            mxT_s = sp.tile([K, K], F32)
            myT_s = sp.tile([K, K], F32)
            ev_t = sp.tile([1, 2 * K], F32)
            nc.sync.dma_start(mx_s, mx_d[:, :])
            nc.sync.dma_start(my_s, my_d[:, :])
            nc.sync.dma_start(mxT_s, mxT_d[:, :])
            nc.sync.dma_start(myT_s, myT_d[:, :])
            nc.sync.dma_start(ev_t, ev_d[:, :])

            ident = sp.tile([C, C], F32)
            make_identity(nc, ident)
            id64 = ident[0:K, 0:K]
            ones_row = sp.tile([1, K], F32)
            nc.vector.memset(ones_row, 1.0)
            ones_col = sp.tile([K, 1], F32)
            nc.vector.memset(ones_col, 1.0)


            # ---------------- projections: AT = fx^T pxT, ByT = fy^T pyT ----
            with tc.tile_pool(name="pacc", bufs=1, space="PSUM") as pacc:
                at_p = pacc.tile([C, K], F32)    # A^T partial  [C,K]
                byt_p = pacc.tile([C, K], F32)   # By^T partial [C,K]
                for n in range(n_chunks):
                    nc.tensor.matmul(at_p, fx_t[:, n, :], pxT_t[:, n, :],
                                     start=(n == 0), stop=(n == n_chunks - 1))
                for n in range(n_chunks):
                    nc.tensor.matmul(byt_p, fy_t[:, n, :], pyT_t[:, n, :],
                                     start=(n == 0), stop=(n == n_chunks - 1))

                if shard:
                    part_s = sp.tile([C, 2 * K], F32)
                    nc.vector.tensor_copy(part_s[:, 0:K], at_p)
                    nc.vector.tensor_copy(part_s[:, K:2 * K], byt_p)
                    nc.sync.dma_start(cc_in[:, :], part_s)
                    nc.gpsimd.collective_compute(
                        "AllReduce",
                        mybir.AluOpType.add,
                        replica_groups=[list(range(N_CORES))],
                        ins=[cc_in[:, :]],
                        outs=[cc_out[:, :]],
                    )
                else:
                    at_s = sb_copy(at_p, [C, K], sp, "at_s")
                    byt_s = sb_copy(byt_p, [C, K], sp, "byt_s")

            # ------- collective-independent work first (hides CC latency) ---
            # G = My^T My
            g_p = ps_tile([K, K])
            nc.tensor.matmul(g_p, my_s, my_s)
            g_s = sb_copy(g_p, [K, K], sp, "g_s")

            # resolvent masks: ev = [ex | ey]; t = ev/max(ev); im = 1/(1+t);
            # re = sqrt(t)*im; both scaled by sqrt(LMBDA)
            evmax = sp.tile([1, 1], F32)
            nc.vector.tensor_reduce(evmax, ev_t, mybir.AxisListType.X,
                                    mybir.AluOpType.max)
            evrec = sp.tile([1, 1], F32)
            nc.vector.reciprocal(evrec, evmax)
            t_t = sp.tile([1, 2 * K], F32)
            nc.vector.tensor_scalar_mul(t_t, ev_t, evrec)
            tp1 = sp.tile([1, 2 * K], F32)
            nc.vector.tensor_scalar_add(tp1, t_t, 1.0)
            im_t = sp.tile([1, 2 * K], F32)
            nc.vector.reciprocal(im_t, tp1)
            sq_t = sp.tile([1, 2 * K], F32)
            nc.scalar.sqrt(sq_t, t_t)
            re_t = sp.tile([1, 2 * K], F32)
            nc.vector.tensor_mul(re_t, sq_t, im_t)
            nc.vector.tensor_scalar_mul(re_t, re_t, SQRT_LMBDA)
            nc.vector.tensor_scalar_mul(im_t, im_t, SQRT_LMBDA)

            # D1T[a,i] = re2[a] - re1[i]; D2T likewise from im
            d_s = []
            for idx, src in enumerate((re_t, im_t)):
                pa = ps_tile([K, K])
                nc.tensor.matmul(pa, src[0:1, K:2 * K], ones_row)  # v2[p]
                pb = ps_tile([K, K])
                nc.tensor.matmul(pb, ones_row, src[0:1, 0:K])      # v1[f]
                ta = sb_copy(pa, [K, K], sp, f"dta{idx}")
                dt = sp.tile([K, K], F32, tag=f"d{idx}t_s", name=f"d{idx}t_s")
                nc.vector.tensor_sub(dt, ta, pb)
                d_s.append(dt)
            d1t_s, d2t_s = d_s

            # Newton-Schulz inverse (S symmetric PD): X' = 2X - X S X
            def newton_inverse(mat_s, tag, steps):
                rs = sp.tile([K, 1], F32, tag=f"{tag}_rs", name=f"{tag}_rs")
                nc.vector.tensor_reduce(rs, mat_s, mybir.AxisListType.X,
                                        mybir.AluOpType.add,
                                        apply_absolute_value=True)
                nc.gpsimd.partition_all_reduce(rs, rs, K, ReduceOp.max)
                al = sp.tile([K, 1], F32, tag=f"{tag}_al", name=f"{tag}_al")
                nc.vector.reciprocal(al, rs)
                x_s = sp.tile([K, K], F32, tag=f"{tag}_x0", name=f"{tag}_x0")
                nc.vector.tensor_scalar_mul(x_s, id64, al)
                for it in range(steps):
                    t1 = ps_tile([K, K])
                    nc.tensor.matmul(t1, mat_s, x_s)          # S X (S sym)
                    t1s = wp.tile([K, K], F32, tag=f"{tag}_t1s",
                                  name=f"{tag}_t1s")
                    nc.vector.tensor_copy(t1s, t1)
                    t2 = ps_tile([K, K])
                    nc.tensor.matmul(t2, x_s, t1s)            # X (S X) (X sym)
                    xn = sp.tile([K, K], F32, tag=f"{tag}_x{it + 1}",
                                 name=f"{tag}_x{it + 1}")
                    nc.vector.scalar_tensor_tensor(
                        xn, x_s, 2.0, t2,
                        op0=mybir.AluOpType.mult,
                        op1=mybir.AluOpType.subtract)
                    x_s = xn
                return x_s

            gi_s = newton_inverse(g_s, "gi", NEWTON_STEPS_G)

            # ------- collective-dependent chain -----------------------------
            if shard:
                full_s = sp.tile([C, 2 * K], F32)
                nc.sync.dma_start(full_s, cc_out[:, :])
                at_s = full_s[:, 0:K]
                byt_s = full_s[:, K:2 * K]

            # S~ = Mx^T (A A^T) Mx    [S_A symmetric -> no transpose]
            sa_p = ps_tile([K, K])
            nc.tensor.matmul(sa_p, at_s, at_s)          # A A^T
            sa_s = sb_copy(sa_p, [K, K], sp, "sa_s")
            h1t_p = ps_tile([K, K])
            nc.tensor.matmul(h1t_p, sa_s, mx_s)         # S_A Mx (sym trick)
            h1t_s = sb_copy(h1t_p, [K, K], sp, "h1t_s")
            st_p = ps_tile([K, K])
            nc.tensor.matmul(st_p, mx_s, h1t_s)         # Mx^T S_A Mx
            st_s = sb_copy(st_p, [K, K], sp, "st_s")

            # RHS' = My^T Bc A^T Mx = My^T (My (By A^T)) Mx
            byat_p = ps_tile([K, K])
            nc.tensor.matmul(byat_p, byt_s, at_s)       # By A^T
            byat_s = sb_copy(byat_p, [K, K], sp, "byat_s")
            bca_p = ps_tile([K, K])
            nc.tensor.matmul(bca_p, myT_s, byat_s)      # My (By A^T) = Bc A^T
            bca_s = sb_copy(bca_p, [K, K], sp, "bca_s")
            w_p = ps_tile([K, K])
            nc.tensor.matmul(w_p, my_s, bca_s)          # My^T Bc A^T
            w_s = sb_copy(w_p, [K, K], sp, "w_s")
            wt_p = ps_tile([K, K])
            nc.tensor.transpose(wt_p, w_s, id64)
            wt_s = sb_copy(wt_p, [K, K], sp, "wt_s")
            rp_p = ps_tile([K, K])
            nc.tensor.matmul(rp_p, wt_s, mx_s)          # (My^T Bc A^T) Mx
            r_s = sp.tile([K, K], F32)                  # CG residual
            nc.vector.tensor_copy(r_s, rp_p)

            si_s = newton_inverse(st_s, "si", NEWTON_STEPS_S)

            # ---------------- PCG (transposed space, transpose-free) --------
            y_s = sp.tile([K, K], F32)
            nc.vector.memset(y_s, 0.0)
            p_s = sp.tile([K, K], F32)
            u_s = sp.tile([K, 2 * K], F32)   # stacked [D1T*p | D2T*p]

            def precond(r_tile):
                """z = (Gi r) Si via (Gi r)^T = mm(lhsT=r, rhs=Gi)."""
                ut_p = ps_tile([K, K])
                nc.tensor.matmul(ut_p, r_tile, gi_s)          # (Gi r)^T
                ut_s = wp.tile([K, K], F32, tag="pc_uts", name="pc_uts")
                nc.scalar.copy(ut_s, ut_p)
                z_p = ps_tile([K, K])
                nc.tensor.matmul(z_p, ut_s, si_s)             # (Gi r) Si
                return z_p

            def dot_b(a_ap, b_ap, tag):
                """<a,b> broadcast to all partitions as [K,1] SBUF."""
                prod = wp.tile([K, K], F32, tag="dot_dm", name="dot_dm")
                acc = wp.tile([K, 1], F32, tag=f"{tag}_acc", name=f"{tag}_acc")
                nc.vector.scalar_tensor_tensor(
                    prod, a_ap, 1.0, b_ap,
                    op0=mybir.AluOpType.bypass, op1=mybir.AluOpType.mult,
                    accum_out=acc)
                nc.gpsimd.partition_all_reduce(acc, acc, K, ReduceOp.add)
                return acc

            z_p = precond(r_s)
            z_s = sp.tile([K, K], F32)
            nc.vector.tensor_copy(z_s, z_p)
            nc.vector.tensor_copy(p_s, z_p)
            rz0 = dot_b(r_s, z_p, "rz")
            rzrec = wp.tile([K, 1], F32, tag="rzrec", name="rzrec")
            nc.vector.reciprocal(rzrec, rz0)
            rzneg = wp.tile([K, 1], F32, tag="rzneg", name="rzneg")
            nc.vector.tensor_scalar_mul(rzneg, rz0, -1.0)

            for it in range(N_ITERS):
                # ---- q = (G p) S~ + sum_d DdT*(G(DdT*p)) ----
                nc.vector.tensor_mul(u_s[:, 0:K], d1t_s, p_s)
                nc.vector.tensor_mul(u_s[:, K:2 * K], d2t_s, p_s)
                gpt_p = ps_tile([K, K])
                nc.tensor.matmul(gpt_p, p_s, g_s)         # (G p)^T
                gpt_s = wp.tile([K, K], F32, tag="mv_gpts", name="mv_gpts")
                nc.scalar.copy(gpt_s, gpt_p)
                t2_p = ps_tile([K, K])
                nc.tensor.matmul(t2_p, gpt_s, st_s)       # (G p) S~
                gu_p = ps_tile([K, 2 * K])
                nc.tensor.matmul(gu_p, g_s, u_s)          # [G u1 | G u2]
                m1_s = wp.tile([K, K], F32, tag="mv_m1", name="mv_m1")
                nc.vector.tensor_mul(m1_s, d1t_s, gu_p[:, 0:K])
                m2_s = wp.tile([K, K], F32, tag="mv_m2", name="mv_m2")
                nc.vector.tensor_mul(m2_s, d2t_s, gu_p[:, K:2 * K])
                ms_s = wp.tile([K, K], F32, tag="mv_ms", name="mv_ms")
                nc.vector.tensor_add(ms_s, m1_s, m2_s)
                q_s = wp.tile([K, K], F32, tag="mv_q", name="mv_q")
                nc.vector.tensor_add(q_s, ms_s, t2_p)

                # ---- alpha = rz/<p,q>; y += alpha p; r -= alpha q ----
                # zq = P^-1 q runs on PE concurrently with the <p,q> dot, so
                # the preconditioner is off the critical path (z, rz still
                # exact: z updated by recurrence z -= alpha zq, rz by dot).
                pq = dot_b(p_s, q_s, "pq")
                if it < N_ITERS - 1:
                    zq_p = precond(q_s)
                pqr = wp.tile([K, 1], F32, tag="pqr", name="pqr")
                nc.vector.reciprocal(pqr, pq)
                al = wp.tile([K, 1], F32, tag="al", name="al")
                nc.vector.tensor_mul(al, rz0, pqr)
                an = wp.tile([K, 1], F32, tag="an", name="an")
                nc.vector.tensor_mul(an, rzneg, pqr)
                nc.vector.scalar_tensor_tensor(
                    y_s, p_s, al, y_s,
                    op0=mybir.AluOpType.mult, op1=mybir.AluOpType.add)
                nc.vector.scalar_tensor_tensor(
                    r_s, q_s, an, r_s,
                    op0=mybir.AluOpType.mult, op1=mybir.AluOpType.add)

                if it == N_ITERS - 1:
                    break

                # ---- z -= alpha zq; beta = <r,z>/rz; p = beta p + z ----
                nc.vector.scalar_tensor_tensor(
                    z_s, zq_p, an, z_s,
                    op0=mybir.AluOpType.mult, op1=mybir.AluOpType.add)
                rz_new = dot_b(r_s, z_s, "rz")
                bt = wp.tile([K, 1], F32, tag="bt", name="bt")
                nc.vector.tensor_mul(bt, rz_new, rzrec)
                nc.vector.scalar_tensor_tensor(
                    p_s, p_s, bt, z_s,
                    op0=mybir.AluOpType.mult, op1=mybir.AluOpType.add)
                rz0 = rz_new
                rzrec = wp.tile([K, 1], F32, tag="rzrec", name="rzrec")
                nc.vector.reciprocal(rzrec, rz0)
                rzneg = wp.tile([K, 1], F32, tag="rzneg", name="rzneg")
                nc.vector.tensor_scalar_mul(rzneg, rz0, -1.0)

            # ---------------- output: C = Y Mx^T ----------------
            yt_p = ps_tile([K, K])
            nc.tensor.transpose(yt_p, y_s, id64)
            yt_s = wp.tile([K, K], F32, tag="yt_s", name="yt_s")
            nc.vector.tensor_copy(yt_s, yt_p)
            c_p = ps_tile([K, K])
            nc.tensor.matmul(c_p, yt_s, mxT_s)      # Y Mx^T
            c_s = wp.tile([K, K], F32, tag="c_s", name="c_s")
            nc.vector.tensor_copy(c_s, c_p)
            nc.sync.dma_start(out_d[:, :], c_s)

    nc.finalize()
    return nc


def get_program(shard: bool):
    if shard not in _PROGRAM_CACHE:
        _PROGRAM_CACHE[shard] = build_program(shard)
    return _PROGRAM_CACHE[shard]


def make_in_maps(inputs, shard: bool):
    fx = np.ascontiguousarray(np.asarray(inputs["feat_x"], np.float32)[0])
    fy = np.ascontiguousarray(np.asarray(inputs["feat_y"], np.float32)[0])
    pxT = np.ascontiguousarray(np.asarray(inputs["evecs_trans_x"], np.float32)[0].T)
    pyT = np.ascontiguousarray(np.asarray(inputs["evecs_trans_y"], np.float32)[0].T)
    mx = np.ascontiguousarray(np.asarray(inputs["sqrtMk_x"], np.float32)[0])
    my = np.ascontiguousarray(np.asarray(inputs["sqrtMk_y"], np.float32)[0])
    ev = np.ascontiguousarray(np.concatenate([
        np.asarray(inputs["evals_x"], np.float32)[0],
        np.asarray(inputs["evals_y"], np.float32)[0],
    ])[None, :])
    small = {
        "mx": mx, "my": my,
        "mxT": np.ascontiguousarray(mx.T),
        "myT": np.ascontiguousarray(my.T),
        "ev": ev,
    }
    in_maps = []
    for c in range(N_CORES):
        if shard:
            lo, hi = c * (V // N_CORES), (c + 1) * (V // N_CORES)
            m = {"fx": fx[lo:hi], "fy": fy[lo:hi],
                 "pxT": pxT[lo:hi], "pyT": pyT[lo:hi]}
        else:
            m = {"fx": fx, "fy": fy, "pxT": pxT, "pyT": pyT}
        m.update(small)
        in_maps.append(m)
    return in_maps


def kernel(**inputs) -> np.ndarray:
    nc = get_program(SHARD)
    in_maps = make_in_maps(inputs, SHARD)
    res = run_bass_kernel_spmd(nc, in_maps, core_ids=list(range(N_CORES)))
    out = np.asarray(res.results[0]["out"], dtype=np.float32)
    return out[None]
